# revision 2
# baseline (speedup 1.0000x reference)
"""Trainium2 Bass kernel for the ExemplarModel (Mahalanobis-kNN attention).

Reference math (N=1024 queries, M=50000 exemplars, D=512, C=10 classes):
    dist[n,m]  = sum_d Sigma_inv[d] * (x[n,d] - e[m,d])^2
    att[n,m]   = exp(-beta * dist[n,m])
    logits[n,c]= segment_sum(att over exemplars with label c)
    out        = softmax(gamma * logits, axis=1)

Distribution: exemplars/labels are sharded along M across 8 NeuronCores
(6250 each, zero-padded to 6272 = 49*128); x, Sigma_inv, beta are
replicated.  Each core computes partial per-class logits
    P[c,n] = sum_m onehot[m,c] * exp(2*beta*cross[n,m] - beta*e_sq[m])
with cross = (x*sqrtS) @ (e*sqrtS)^T, via
  - a bf16 TensorE matmul for cross (K=512 contraction in 4 chunks),
  - ScalarE Exp reading cross straight out of PSUM, with the per-exemplar
    -beta*e_sq folded into the activation bias,
  - a second TensorE matmul against the per-shard one-hot label matrix,
    accumulated in PSUM across all 49 exemplar tiles.
The host then combines: logits[n,c] = exp(-beta*x_sq[n]) * sum_cores P,
and applies gamma + softmax on the tiny [1024,10] result.
"""

import numpy as np

import concourse.bass as bass
import concourse.bacc as bacc
import concourse.tile as tile
from concourse import mybir
from concourse import bass_utils

# Problem constants (hardcoded per contract; kernel.py must be self-contained).
N = 1024          # queries
M = 50000         # exemplars (global)
D = 512           # feature dim
C = 10            # classes
N_CORES = 8
M_LOC = M // N_CORES          # 6250 exemplars per core
P = 128                       # partitions
T_TILES = (M_LOC + P - 1) // P  # 49 tiles per core
M_PAD = T_TILES * P           # 6272
KC = D // P                   # 4 contraction chunks
NH = N // 512                 # 2 matmul free-dim halves

FP32 = mybir.dt.float32
BF16 = mybir.dt.bfloat16


def build_nc(t_tiles=T_TILES, n=N, debug=False):
    """Build the per-core Bass program (SPMD: same program, per-core data)."""
    nc = bacc.Bacc("TRN2", target_bir_lowering=False, debug=debug,
                   num_devices=N_CORES)
    m_pad = t_tiles * P
    nh = n // 512

    e_dram = nc.dram_tensor("e", [m_pad, D], FP32, kind="ExternalInput")
    w_dram = nc.dram_tensor("w", [P, t_tiles * C], FP32, kind="ExternalInput")
    xsT_dram = nc.dram_tensor("xsT", [D, n], FP32, kind="ExternalInput")
    sbc_dram = nc.dram_tensor("sbc", [P, D], FP32, kind="ExternalInput")
    negb_dram = nc.dram_tensor("negb", [P, 1], FP32, kind="ExternalInput")
    twob_dram = nc.dram_tensor("twob", [P, 1], FP32, kind="ExternalInput")
    out_dram = nc.dram_tensor("out", [C, n], FP32, kind="ExternalOutput")

    with tile.TileContext(nc) as tc:
        with (
            tc.tile_pool(name="const", bufs=1) as const_pool,
            tc.tile_pool(name="e_in", bufs=4) as e_pool,
            tc.tile_pool(name="es", bufs=4) as es_pool,
            tc.tile_pool(name="esT", bufs=4) as esT_pool,
            tc.tile_pool(name="small", bufs=4) as small_pool,
            tc.tile_pool(name="att", bufs=4) as att_pool,
            tc.tile_pool(name="crossp", bufs=2, space="PSUM") as cross_pool,
            tc.tile_pool(name="logitp", bufs=1, space="PSUM") as logit_pool,
        ):
            # ---- one-time preamble ----
            xsT_f32 = const_pool.tile([P, KC * n], FP32, tag="xsTf")
            for k in range(KC):
                nc.gpsimd.dma_start(xsT_f32[:, k * n:(k + 1) * n],
                                    xsT_dram[k * P:(k + 1) * P, :])
            xsT_bf = const_pool.tile([P, KC * n], BF16, tag="xsTb")
            nc.vector.tensor_copy(xsT_bf[:], xsT_f32[:])

            w_f32 = const_pool.tile([P, t_tiles * C], FP32, tag="wf")
            nc.gpsimd.dma_start(w_f32[:], w_dram[:])
            w_bf = const_pool.tile([P, t_tiles * C], BF16, tag="wb")
            nc.vector.tensor_copy(w_bf[:], w_f32[:])

            sbc = const_pool.tile([P, D], FP32, tag="sbc")
            nc.gpsimd.dma_start(sbc[:], sbc_dram[:])
            negb = const_pool.tile([P, 1], FP32, tag="negb")
            nc.gpsimd.dma_start(negb[:], negb_dram[:])
            twob = const_pool.tile([P, 1], FP32, tag="twob")
            nc.gpsimd.dma_start(twob[:], twob_dram[:])

            logits_ps = logit_pool.tile([C, n], FP32)

            # ---- main loop over exemplar tiles ----
            for t in range(t_tiles):
                e_t = e_pool.tile([P, D], FP32, tag="e")
                nc.gpsimd.dma_start(e_t[:], e_dram[t * P:(t + 1) * P, :])

                # es = e * sqrt(Sigma_inv), cast to bf16
                es_t = es_pool.tile([P, D], BF16, tag="es")
                nc.vector.tensor_tensor(es_t[:], e_t[:], sbc[:],
                                        mybir.AluOpType.mult)

                # e_sq[m] = sum_d es^2
                # (tensor_tensor_reduce is not supported on this HW path)
                es2 = es_pool.tile([P, D], BF16, tag="es2")
                nc.vector.tensor_tensor(es2[:], es_t[:], es_t[:],
                                        mybir.AluOpType.mult)
                esq = small_pool.tile([P, 1], FP32, tag="esq")
                nc.vector.tensor_reduce(esq[:], es2[:], mybir.AxisListType.X,
                                        mybir.AluOpType.add)

                # bias = -beta * e_sq
                bias_t = small_pool.tile([P, 1], FP32, tag="bias")
                nc.vector.tensor_scalar(bias_t[:], esq[:], negb[:], None,
                                        mybir.AluOpType.mult)

                # esT chunks via SBUF->SBUF DMA transpose (2-byte dtype)
                esT_t = esT_pool.tile([P, D], BF16, tag="esT")
                for k in range(KC):
                    nc.sync.dma_start(esT_t[:, k * P:(k + 1) * P],
                                      es_t[:, k * P:(k + 1) * P],
                                      transpose=True)

                # cross[m, n] = sum_d es[m,d] * xs[n,d]
                cross_ps = cross_pool.tile([P, n], FP32, tag="cross")
                for k in range(KC):
                    for h in range(nh):
                        nc.tensor.matmul(
                            cross_ps[:, h * 512:(h + 1) * 512],
                            lhsT=esT_t[:, k * P:(k + 1) * P],
                            rhs=xsT_bf[:, k * n + h * 512: k * n + h * 512 + 512],
                            start=(k == 0), stop=(k == KC - 1))

                # att = exp(2*beta*cross - beta*e_sq)  (ACT reads PSUM)
                att_t = att_pool.tile([P, n], BF16, tag="att")
                nc.scalar.activation(att_t[:], cross_ps[:],
                                     mybir.ActivationFunctionType.Exp,
                                     bias=bias_t[:], scale=twob[:])

                # logits[c, n] += onehot[m, c]^T @ att[m, n]
                for h in range(nh):
                    nc.tensor.matmul(
                        logits_ps[:, h * 512:(h + 1) * 512],
                        lhsT=w_bf[:, t * C:(t + 1) * C],
                        rhs=att_t[:, h * 512:(h + 1) * 512],
                        start=(t == 0), stop=(t == t_tiles - 1),
                        skip_group_check=True)

            # ---- epilogue ----
            out_sb = const_pool.tile([C, n], FP32, tag="out")
            nc.vector.tensor_copy(out_sb[:], logits_ps[:])
            nc.gpsimd.dma_start(out_dram[:], out_sb[:])

    nc.compile()
    return nc


def make_in_maps(x, exemplars, labels, Sigma_inv, beta, gamma,
                 t_tiles=T_TILES):
    """Shard the full inputs into per-core in_maps (host-side glue)."""
    x = np.asarray(x, dtype=np.float32)
    exemplars = np.asarray(exemplars, dtype=np.float32)
    labels = np.asarray(labels).astype(np.int64)
    Sigma_inv = np.asarray(Sigma_inv, dtype=np.float32)
    beta = float(np.asarray(beta).reshape(-1)[0])

    m_pad = t_tiles * P
    n = x.shape[0]
    sqrtS = np.sqrt(Sigma_inv).astype(np.float32)
    xsT = np.ascontiguousarray((x * sqrtS).T)            # [D, N]
    sbc = np.ascontiguousarray(np.broadcast_to(sqrtS, (P, D)))
    negb = np.full((P, 1), -beta, dtype=np.float32)
    twob = np.full((P, 1), 2.0 * beta, dtype=np.float32)

    m_loc = M // N_CORES
    in_maps = []
    for c in range(N_CORES):
        e_shard = np.zeros((m_pad, D), dtype=np.float32)
        e_shard[:m_loc] = exemplars[c * m_loc:(c + 1) * m_loc]
        lab = labels[c * m_loc:(c + 1) * m_loc]
        onehot = np.zeros((m_pad, C), dtype=np.float32)
        onehot[np.arange(m_loc), lab] = 1.0
        w_packed = np.ascontiguousarray(
            onehot.reshape(t_tiles, P, C).transpose(1, 0, 2).reshape(P, t_tiles * C))
        in_maps.append({
            "e": e_shard, "w": w_packed, "xsT": xsT, "sbc": sbc,
            "negb": negb, "twob": twob,
        })
    return in_maps


def finalize(core_outs, x, Sigma_inv, beta, gamma):
    """Combine per-core partial logits into the full softmax output."""
    x = np.asarray(x, dtype=np.float32)
    Sigma_inv = np.asarray(Sigma_inv, dtype=np.float32)
    beta = float(np.asarray(beta).reshape(-1)[0])
    gamma = float(np.asarray(gamma).reshape(-1)[0])

    partial = np.zeros_like(core_outs[0], dtype=np.float32)
    for o in core_outs:
        partial += o                                      # [C, N]
    x_sq = np.einsum("nd,d->n", x * x, Sigma_inv)         # [N]
    logits = np.exp(-beta * x_sq)[:, None].astype(np.float32) * partial.T
    z = gamma * logits
    z = z - z.max(axis=1, keepdims=True)
    ez = np.exp(z)
    return (ez / ez.sum(axis=1, keepdims=True)).astype(np.float32)


_NC_CACHE = {}


def kernel(x, exemplars, labels, Sigma_inv, beta, gamma):
    if "nc" not in _NC_CACHE:
        _NC_CACHE["nc"] = build_nc()
    nc = _NC_CACHE["nc"]
    in_maps = make_in_maps(x, exemplars, labels, Sigma_inv, beta, gamma)
    res = bass_utils.run_bass_kernel_spmd(nc, in_maps,
                                          core_ids=list(range(N_CORES)))
    core_outs = [r["out"] for r in res.results]
    return finalize(core_outs, x, Sigma_inv, beta, gamma)
